# revision 3
# baseline (speedup 1.0000x reference)
"""Trainium2 Bass kernel for nn_GAT_87617332838818.

Mathematical collapse: the reference GAT aggregates ``alpha * hp[:, dst]``
over incoming edges per destination node.  Since the softmax weights alpha
sum to exactly 1 within each destination segment and the aggregated message
``hp[dst]`` is constant within the segment, the whole message-passing step
is the identity: ``out[n] = hp[n]``.  The network therefore reduces to a
per-node 3-layer MLP:

    logits = W2r @ elu(W1r @ elu(W0r @ x^T))        (per node column)

with W0r = W0.reshape(96,128), W1r = W1.reshape(96,96), W2r = W2.reshape(40,96).

Device strategy (8 NeuronCores, node-sharded 6250 rows each), v2:
  - exact bias-free ELU:  elu(p) = max(p,0) + (min(exp(p),1) - 1).
    r = max(p,0) (1x PSUM pass) and t' = min(e,1)-1 (DVE MIN,ADD 2-op
    form: 4x fast path; the 1-op MIN,BYPASS form hits a pathological
    DVE slow path ~30x worse - measured) feed TWO accumulating matmuls.
  - 1024-col super-groups (2 PSUM banks per tile; engine APs may cross
    banks - verified on HW): halves instruction and semaphore counts vs
    512-col pairs.  Groups: 6x1024 + 106 remainder (remainder last for
    a short pipeline tail).
  - static SBUF allocation (unique tag per tensor instance, bufs=1):
    no write-after-read reuse deps -> far fewer EVENT_SEMAPHORE stalls
    (baseline burned ~8.5us/engine on sem waits).
  - PSUM: one shared pool bufs=3 x [96,1024] rotating L0/L1 tiles, plus
    single-buffered [104,1024] L2 tile packing TWO groups per drain
    (rows 0:40 / 64:104; PSUM matmul base partitions 0/64).
  - drain work split ACT/DVE by measured rates (ACT 1.2GHz vs DVE
    0.96GHz; exp is ACT-only): ACT = exp + a few relus + some out
    drains, DVE = remaining relus + all t-passes + rest of out drains.
  - dummy matmuls parked in the DMA-bound head flip the PE clock to
    2.4 GHz before the real matmuls start (see baseline notes: 15 junk
    matmuls measured best; fewer makes the real stream start too dense
    and the clock governor drops it to 1.2 GHz).
  - w0 rides in the first x DMA batch; w1/w2 pack into one DMA.
"""

import os
import sys

import numpy as np

for _p in ("/root/.axon_site/_ro/trn_rl_repo", "/opt/trn_rl_repo"):
    if os.path.isdir(_p) and _p not in sys.path:
        sys.path.append(_p)

import concourse.bass as bass
import concourse.tile as tile
from concourse import bacc, mybir
from concourse.bass_utils import run_bass_kernel_spmd

N_CORES = 8
N_PER = 6250            # 50000 / 8
D_IN = 128
D_HID = 96
D_OUT = 40
MM_N = 512              # matmul moving free-dim (1 PSUM bank)
GRP = 1024              # super-group free-dim (2 PSUM banks)

F16 = mybir.dt.float16
BF16 = mybir.dt.bfloat16
F32 = mybir.dt.float32

Act = mybir.ActivationFunctionType
Alu = mybir.AluOpType

_groups = [GRP] * (N_PER // GRP)
if N_PER % GRP:
    _groups.append(N_PER % GRP)
G = len(_groups)                                  # 7
_gstarts = [sum(_groups[:i]) for i in range(G)]
N_PAIRS = (G + 1) // 2                            # 4 (last is solo 106)
# pairs p<3 at yT cols [1024p,1024p+1024) rows 0:40/64:104; solo g=6 at
# cols [3072,3178) rows 0:40
YT_COLS = (G // 2) * GRP + (_groups[-1] if G % 2 else 0)   # 3178

# which relu drains go on ACT instead of DVE, by (layer, group).
# Balance solve with measured rates (exp 1110/relu 1110|1220/t 380/out
# 1003|1130 per 1024 cols): ACT = exp(13.8) + 3 relu + 1 out ~= 18.3us,
# DVE = t(4.9) + 9 relu + rems + 2.3 out ~= 18.3us.
_env = os.environ.get
RELU_ON_ACT = {
    tuple(int(c) for c in s.split("."))
    for s in _env("GAT_ACT_RELU", "0.1,0.3,1.4").split(",") if s
}
OUT_ON_ACT = {int(s) for s in _env("GAT_ACT_OUT", "1").split(",") if s}
N_WARMUP_MM = int(_env("GAT_WARMUP", "15"))

# x DMA batches, in groups: batch 0 carries w0 + group 0
X_BATCHES = [1, 2, 2, 2]
_batch_of = {}
_b0 = 0
for _bi, _bn in enumerate(X_BATCHES):
    for _g in range(_b0, min(_b0 + _bn, G)):
        _batch_of[_g] = _bi
    _b0 += _bn
assert _b0 >= G


def _mm_splits(fd):
    out = []
    j = 0
    while j < fd:
        out.append((j, min(j + MM_N, fd)))
        j += MM_N
    return out


def _build_program() -> bass.Bass:
    nc = bacc.Bacc(None, target_bir_lowering=False, debug=False)

    # xw packs [w0t | xT]: cols 0..95 = W0^T fp16, cols 96.. = x^T shard
    xw = nc.declare_dram_parameter("xw", [D_IN, D_HID + N_PER], F16,
                                   isOutput=False)
    # wb packs [w1t | w2t] bf16
    wb = nc.declare_dram_parameter("wb", [D_HID, D_HID + D_OUT], BF16,
                                   isOutput=False)
    # packed output: pair p at cols [1024p, 1024p+1024): rows 0:40 = group
    # 2p, rows 64:104 = group 2p+1; solo group 6 at cols 3072:3178 rows
    # 0:40.  Host unpacks.
    yT = nc.declare_dram_parameter("yT", [104, YT_COLS], F16, isOutput=True)

    st = {}
    st_batch = {}
    shared = {}

    with tile.TileContext(nc) as tc:
        with (
            tc.tile_pool(name="consts", bufs=1) as consts,
            tc.tile_pool(name="xin", bufs=1) as xpool,
            tc.tile_pool(name="sb", bufs=1) as sb,
            tc.tile_pool(name="ps", bufs=3, space="PSUM") as ps,
            tc.tile_pool(name="ps2", bufs=1, space="PSUM") as ps2,
        ):
            # --- PE warm-up on garbage SBUF during the DMA-bound head.
            junk_w = consts.tile([D_IN, D_OUT], F16, tag="junkw")
            junk_x = consts.tile([D_IN, MM_N], F16, tag="junkx")
            nc.gpsimd.memset(junk_w[:], 0.0)
            nc.gpsimd.memset(junk_x[:], 0.0)
            warm = ps2.tile([104, GRP], F32, tag="p2")
            for _ in range(N_WARMUP_MM):
                nc.tensor.matmul(warm[:D_OUT, :MM_N], junk_w[:], junk_x[:],
                                 start=True, stop=True)

            wb_sb = consts.tile([D_HID, D_HID + D_OUT], BF16, tag="wb")
            w1_sb = wb_sb[:, :D_HID]
            w2_sb = wb_sb[:, D_HID:D_HID + D_OUT]

            def stage_load(g):
                bi = _batch_of[g]
                if g > 0 and _batch_of[g - 1] == bi:
                    st[g] = st_batch[bi]
                    return
                g1 = g
                while g1 + 1 < G and _batch_of[g1 + 1] == bi:
                    g1 += 1
                lo = _gstarts[g] + (0 if bi else -D_HID)   # batch 0 incl. w0
                hi = _gstarts[g1] + _groups[g1]
                cols = hi - lo
                xt = xpool.tile([D_IN, cols], F16, tag=f"xt{bi}")
                nc.sync.dma_start(xt[:, :cols], xw[:, D_HID + lo:D_HID + hi])
                st_batch[bi] = {"xt": xt, "base": lo}
                st[g] = st_batch[bi]

            def l0_mm(g):
                fd = _groups[g]
                s = dict(st[g])
                st[g] = s
                xo = _gstarts[g] - s["base"]
                w0_sb = shared["w0"]
                p0 = ps.tile([D_HID, GRP], F32, tag="p")
                for j0, j1 in _mm_splits(fd):
                    nc.tensor.matmul(p0[:, j0:j1], w0_sb,
                                     s["xt"][:, xo + j0:xo + j1],
                                     start=True, stop=True)
                s["p0"] = p0

            def drain(g, lyr, psum):
                """exp + relu + t' from psum[96, fd]; returns (r, t)."""
                fd = _groups[g]
                e = sb.tile([D_HID, fd], BF16, tag=f"e{lyr}_{g}")
                r = sb.tile([D_HID, fd], BF16, tag=f"r{lyr}_{g}")
                t = sb.tile([D_HID, fd], BF16, tag=f"t{lyr}_{g}")
                nc.scalar.activation(e[:, :fd], psum[:, :fd], Act.Exp)
                if (lyr, g) in RELU_ON_ACT:
                    nc.scalar.activation(r[:, :fd], psum[:, :fd], Act.Relu)
                else:
                    nc.vector.tensor_scalar_max(r[:, :fd], psum[:, :fd], 0.0)
                # 2-op MIN,ADD form: DVE fast path (4x); 1-op MIN is ~30x
                # slower (measured).
                nc.vector.tensor_scalar(t[:, :fd], e[:, :fd], 1.0, -1.0,
                                        Alu.min, Alu.add)
                return r, t

            def l0_drain(g):
                s = st[g]
                s["r0"], s["t0"] = drain(g, 0, s.pop("p0"))

            def l1_mm(g):
                fd = _groups[g]
                s = st[g]
                p1 = ps.tile([D_HID, GRP], F32, tag="p")
                for j0, j1 in _mm_splits(fd):
                    nc.tensor.matmul(p1[:, j0:j1], w1_sb, s["r0"][:, j0:j1],
                                     start=True, stop=False)
                    nc.tensor.matmul(p1[:, j0:j1], w1_sb, s["t0"][:, j0:j1],
                                     start=False, stop=True)
                s["p1"] = p1

            def l1_drain(g):
                s = st[g]
                s["r1"], s["t1"] = drain(g, 1, s.pop("p1"))

            pair_state = {}

            def l2_mm_out(g):
                fd = _groups[g]
                s = st.pop(g)
                kp = g // 2
                if g % 2 == 0:
                    p2 = ps2.tile([104, GRP], F32, tag="p2")
                    pair_state[kp] = p2
                    rows = slice(0, D_OUT)
                else:
                    p2 = pair_state[kp]
                    rows = slice(64, 64 + D_OUT)
                for j0, j1 in _mm_splits(fd):
                    nc.tensor.matmul(p2[rows, j0:j1], w2_sb,
                                     s["r1"][:, j0:j1],
                                     start=True, stop=False)
                    nc.tensor.matmul(p2[rows, j0:j1], w2_sb,
                                     s["t1"][:, j0:j1],
                                     start=False, stop=True)
                if not (g % 2 == 1 or g == G - 1):
                    return
                nrows = 104 if g % 2 == 1 else D_OUT
                o = sb.tile([104, fd], F16, tag=f"o{kp}")
                if kp in OUT_ON_ACT:
                    nc.scalar.activation(o[:nrows, :fd], p2[:nrows, :fd],
                                         Act.Identity)
                else:
                    nc.vector.tensor_copy(o[:nrows, :fd], p2[:nrows, :fd])
                ow = fd if g % 2 == 1 else _groups[g]
                eng = nc.gpsimd if kp % 2 == 0 else nc.sync
                eng.dma_start(yT[:, kp * GRP:kp * GRP + ow], o[:, :ow])

            # 3-stage software-pipelined emission (baseline-proven skew:
            # matmuls a full step after their input drains so each
            # engine's in-order stream always has ready work).
            for k in range(G + 3):
                if k < G:
                    stage_load(k)
                    if k == 0:
                        shared["w0"] = st[0]["xt"][:, 0:D_HID]
                        # consts issue after the first x batch (off the
                        # critical path of the first matmul)
                        nc.gpsimd.dma_start(wb_sb[:], wb[:])
                if 0 <= k - 1 < G:
                    l0_mm(k - 1)
                    l0_drain(k - 1)
                if 0 <= k - 2 < G:
                    l1_mm(k - 2)
                    l1_drain(k - 2)
                if 0 <= k - 3 < G:
                    l2_mm_out(k - 3)

    nc.compile()
    return nc


_prog_cache = []
last_result = None


def kernel(**inputs) -> np.ndarray:
    global last_result
    x = np.asarray(inputs["x"], np.float32)           # [50000, 128]
    W0 = np.asarray(inputs["W0"], np.float32).reshape(D_HID, D_IN)
    W1 = np.asarray(inputs["W1"], np.float32).reshape(D_HID, D_HID)
    W2 = np.asarray(inputs["W2"], np.float32).reshape(D_OUT, D_HID)

    n = x.shape[0]
    assert n == N_CORES * N_PER, f"unexpected node count {n}"

    import ml_dtypes
    xT16 = x.T.astype(np.float16)                            # [128, 50000]
    w0t = W0.T.astype(np.float16)                            # [128, 96]
    w1tb = W1.T.astype(ml_dtypes.bfloat16)                   # [96, 96]
    w2tb = W2.T.astype(ml_dtypes.bfloat16)                   # [96, 40]
    wb = np.ascontiguousarray(
        np.concatenate([w1tb, w2tb], axis=1))                # [96, 136]

    if not _prog_cache:
        _prog_cache.append(_build_program())
    nc = _prog_cache[0]

    in_maps = []
    for i in range(N_CORES):
        xwi = np.ascontiguousarray(
            np.concatenate([w0t, xT16[:, i * N_PER:(i + 1) * N_PER]], axis=1))
        in_maps.append(dict(xw=xwi, wb=wb))
    res = run_bass_kernel_spmd(nc, in_maps, list(range(N_CORES)))
    last_result = res
    out = np.empty((n, D_OUT), np.float32)
    for i in range(N_CORES):
        yt = np.asarray(res.results[i]["yT"], np.float32)  # [104, 3178]
        base = i * N_PER
        for kp in range(N_PAIRS):
            c0 = kp * GRP
            g0 = 2 * kp
            w0_ = _groups[g0]
            out[base + _gstarts[g0]:base + _gstarts[g0] + w0_] = \
                yt[0:D_OUT, c0:c0 + w0_].T
            if g0 + 1 < G:
                w1_ = _groups[g0 + 1]
                out[base + _gstarts[g0 + 1]:base + _gstarts[g0 + 1] + w1_] = \
                    yt[64:64 + D_OUT, c0:c0 + w1_].T
    return out


if __name__ == "__main__":
    data = np.load("/tmp/gat_inputs.npz")
    y = kernel(**{k: data[k] for k in data.files})
    print("out", y.shape, y.dtype, "absmax", np.abs(y).max())
